# revision 35
# baseline (speedup 1.0000x reference)
"""Trainium2 Bass kernel: int8 3x3 VALID conv (1,512,512,32)->(1,510,510,64)
with TFLite fixed-point requantization, SPMD over 8 NeuronCores (output rows).

v7 design:
- Minimal host->device I/O: x ships once per core as int8 [66,32,512];
  the A/B row-phase packings are produced by two strided gather DMAs per
  block and converted int8->bf16 on chip (converts spread over DVE/ACT/
  Pool at [128,512] granularity so the first matmul waits on one group).
- Per half-group: two 1-bank psum tiles (A and B row-pair); 6 matmuls
  ordered j-outer with B first, so consecutive matmuls share stationary
  weights and B's bank closes early.
- Requant (per-channel out = sat(round(acc*sc + zb))): B halves on DVE
  tensor_scalar, A halves on ACT activation -- two independent pipelines
  (separate psum + output tiles, so they never serialize); the final
  half-group stores its two halves via separate DMAs on SP/Pool to
  shorten the tail.
"""
import numpy as np
import ml_dtypes

import concourse.mybir as mybir
import concourse.tile as tile_mod
import concourse.bacc as bacc
from concourse.bass_utils import run_bass_kernel_spmd
from concourse.tile import TileContext
from concourse.ap import AP
from concourse.vector_clock import ScopedClock


def _patched_drain_and_barrier(self, tick_clock, wait_clock):
    # workaround: split the Tile kernel-tail drain into single-wait drains
    # (1 sync-wait per CTRL inst), distributed round-robin across engine
    # queues so they wait in parallel instead of serializing on SP.
    drain_inst = self.nc.sync.drain()
    wait_clock.add_sem_waits(
        drain_inst.ins, ScopedClock({None: tick_clock.global_clock})
    )
    si = drain_inst.ins.sync_info
    if si is not None and si.on_wait and len(si.on_wait) > 1:
        # early-firing clocks first so each engine's drain ladder retires
        # during the final out-DMA wait; DMA-queue clocks (which fire at
        # the last store) land at the ladder ends
        waits = sorted(si.on_wait,
                       key=lambda w: 'DMA' in (getattr(w, 'ant_name', '') or ''))
        drain_inst.ins.sync_info = mybir.SyncInfo(
            on_wait=[waits[0]], on_update=si.on_update
        )
        engines = [self.nc.sync, self.nc.gpsimd, self.nc.vector,
                   self.nc.scalar, self.nc.tensor]
        for i, w in enumerate(waits[1:]):
            d2 = engines[i % len(engines)].drain()
            d2.ins.sync_info = mybir.SyncInfo(on_wait=[w], on_update=[])

    self.nc.all_engine_barrier()
    assert self.sems is not None
    popped = self.nc._tile_sem_poison_stack.pop()
    assert popped is self._sem_poison
    self.nc.clear_and_free_semaphores(list(self.sems.allocated().values()))


tile_mod.TileContext._drain_and_barrier = _patched_drain_and_barrier

dt = mybir.dt
AF = mybir.ActivationFunctionType
OP = mybir.AluOpType

MANT_MAX = 2147418112
H, W, CIN, COUT = 512, 512, 32, 64
WO = 510                     # output width
RC = 64                      # out rows per core
XROWS = 66                   # x rows per core (64 + 2 halo)
NBLK = 4                     # row blocks per core (16 out rows each)
NHG = 16                     # half-groups per core (4 out rows each)


def build_nc(n_cores: int):
    nc = bacc.Bacc('TRN2', target_bir_lowering=False, debug=False,
                   num_devices=n_cores)
    xT = nc.dram_tensor('xT', [XROWS, CIN, W], dt.int8, kind='ExternalInput')
    wgt = nc.dram_tensor('wgt', [128, 3 * 128], dt.bfloat16, kind='ExternalInput')
    qc = nc.dram_tensor('qc', [128, 2], dt.float32, kind='ExternalInput')
    out = nc.dram_tensor('out', [NHG, 128, 2 * WO], dt.int8, kind='ExternalOutput')

    with TileContext(nc) as tc:
        with (
            tc.tile_pool(name='const', bufs=1) as cpool,
            tc.tile_pool(name='xs', bufs=2) as xspool,
            tc.tile_pool(name='ot', bufs=4) as opool,
            tc.tile_pool(name='psa', bufs=4, space='PSUM') as papool,
            tc.tile_pool(name='psb', bufs=3, space='PSUM') as pbpool,
            tc.tile_pool(name='wps', bufs=1, space='PSUM') as wpool,
        ):
            wsb = cpool.tile([128, 3 * 128], dt.bfloat16)
            qsb = cpool.tile([128, 2], dt.float32)
            scr = cpool.tile([128, 512], dt.bfloat16)
            q_sc, q_zb = qsb[:, 0:1], qsb[:, 1:2]

            # head warm-ups, all during the initial DMA wait:
            # - dummy activation pulls the ACT table load (~1.3us) early
            # - dummy matmuls on zeroed data burn the PE clock-ramp
            #   penalty (HAM gate) so real matmuls start at full rate
            #   (dedicated psum pool so no tile dep chains them to DMAs)
            scr2 = cpool.tile([128, 1], dt.float32)
            nc.vector.memset(scr[:], 0.0)
            nc.gpsimd.memset(scr2[:], 0.0)
            nc.scalar.activation(scr2[:], scr2[:], AF.Identity)
            warm = wpool.tile([128, 512], dt.float32)
            for _ in range(4):
                nc.tensor.matmul(warm[:], scr[:, 0:128], scr[:],
                                 start=True, stop=True)

            def xsrc(roff, g0, ng):
                # (p=(q,c), col=(g,w)) <- xT[roff+4(g0+g)+q, c, w]
                return AP(xT, (roff + 4 * g0) * CIN * W,
                          [[CIN * W, 4], [W, 32], [4 * CIN * W, ng], [1, W]])

            for b in range(NBLK):
                sa = xspool.tile([128, 4 * W], dt.int8, tag='sa')
                sb = xspool.tile([128, 4 * W], dt.int8, tag='sb')
                ta = xspool.tile([128, 4 * W], dt.bfloat16, tag='ta')
                tb = xspool.tile([128, 4 * W], dt.bfloat16, tag='tb')
                if b == 0:
                    # critical path first: wgt (ACT queue SEQ overlaps the
                    # table load), then g0 slices of B and A, then the rest
                    nc.scalar.dma_start(wsb[:], wgt[:])
                    nc.gpsimd.dma_start(sb[:, 0:W], xsrc(2, 0, 1))
                    nc.sync.dma_start(sa[:, 0:W], xsrc(0, 0, 1))
                    nc.sync.dma_start(sb[:, W:4 * W], xsrc(2, 1, 3))
                    nc.gpsimd.dma_start(sa[:, W:4 * W], xsrc(0, 1, 3))
                    nc.scalar.dma_start(qsb[:], qc[:])
                else:
                    nc.sync.dma_start(sa[:], xsrc(16 * b, 0, 4))
                    nc.gpsimd.dma_start(sb[:], xsrc(16 * b + 2, 0, 4))
                # int8 -> bf16 at group granularity: tb (feeds the first
                # matmul of each half-group) on DVE; ta on Pool
                for g in range(4):
                    cs = slice(g * W, (g + 1) * W)
                    nc.vector.tensor_scalar(tb[:, cs], sb[:, cs], 0.0, None,
                                            op0=OP.add)
                    nc.gpsimd.tensor_scalar(ta[:, cs], sa[:, cs], 0.0,
                                            None, op0=OP.add)

                for hg in range(4):
                    hgi = 4 * b + hg
                    # independent ACT (A half) and DVE (B half) pipelines:
                    # per-half 1-bank psum tiles; B first so DVE starts a
                    # matmul earlier, and the (B, A) pair per tap j shares
                    # the same stationary weights.
                    ps_b = pbpool.tile([128, 512], dt.float32)
                    ps_a = papool.tile([128, 512], dt.float32)
                    for j in range(3):
                        for t, ps in ((tb, ps_b), (ta, ps_a)):
                            nc.tensor.matmul(
                                ps[:, 0:WO],
                                wsb[:, j * 128:(j + 1) * 128],
                                t[:, hg * W + j: hg * W + j + WO],
                                start=(j == 0), stop=(j == 2))
                    if hgi == NHG - 1:
                        # separate tiles + separate stores so the two
                        # engines' tail chains run fully in parallel
                        ot_a = opool.tile([128, WO], dt.int8, tag='ota')
                        ot_b = opool.tile([128, WO], dt.int8, tag='otb')
                        nc.vector.tensor_scalar(ot_b[:], ps_b[:, 0:WO],
                                                q_sc, q_zb,
                                                op0=OP.mult, op1=OP.add)
                        nc.gpsimd.dma_start(out[hgi, :, WO:2 * WO], ot_b[:])
                        nc.scalar.activation(ot_a[:], ps_a[:, 0:WO],
                                             AF.Identity,
                                             bias=q_zb, scale=q_sc)
                        nc.sync.dma_start(out[hgi, :, 0:WO], ot_a[:])
                    else:
                        # shared ot tile: DVE writes B then ACT writes A;
                        # cross-tile pipelining keeps both engines busy
                        ot = opool.tile([128, 2 * WO], dt.int8, tag='ot')
                        nc.vector.tensor_scalar(ot[:, WO:2 * WO],
                                                ps_b[:, 0:WO],
                                                q_sc, q_zb,
                                                op0=OP.mult, op1=OP.add)
                        nc.scalar.activation(ot[:, 0:WO], ps_a[:, 0:WO],
                                             AF.Identity,
                                             bias=q_zb, scale=q_sc)
                        nc.sync.dma_start(out[hgi], ot[:])
    nc.finalize()
    return nc


def host_prepare(x, filt, bias, q_mantissa, exponent, output_zero_point):
    """Full inputs -> list of per-core in_maps."""
    bf16 = ml_dtypes.bfloat16
    x = np.asarray(x)
    filt = np.asarray(filt)
    bias64 = np.asarray(bias).astype(np.int64)
    qm64 = np.asarray(q_mantissa).astype(np.int64)
    ex64 = np.asarray(exponent).astype(np.int64)
    zp = int(np.asarray(output_zero_point))

    # xT: [H, C, W] int8, padded to 8*64+2 rows
    xpad = np.zeros((8 * RC + 2, CIN, W), dtype=np.int8)
    xpad[:H] = np.ascontiguousarray(x[0].transpose(0, 2, 1))

    # weights: wgt[32q+ci, j, 64a+co] = filt[co, q-a, j, ci] (0 <= q-a <= 2)
    wgtf = np.zeros((128, 3, 128), dtype=np.float32)
    for q in range(4):
        for a in range(2):
            fh = q - a
            if 0 <= fh <= 2:
                wgtf[32 * q:32 * q + 32, :, 64 * a:64 * a + 64] = \
                    filt[:, fh, :, :].transpose(2, 1, 0).astype(np.float32)
    wgt_b = np.ascontiguousarray(wgtf.reshape(128, 384)).astype(bf16)

    # per-channel requant constants (f64 -> f32)
    m = np.where(qm64 < MANT_MAX, (qm64 + (1 << 15)) >> 16, 32767).astype(np.float64)
    s = (15 - ex64).astype(np.float64)
    sc = m * (2.0 ** -s)
    zb = zp + bias64 * sc
    qcv = np.zeros((64, 2), dtype=np.float32)
    qcv[:, 0] = sc
    qcv[:, 1] = zb
    qc128 = np.tile(qcv, (2, 1))

    in_maps = []
    for k in range(8):
        in_maps.append({
            'xT': np.ascontiguousarray(xpad[k * RC: k * RC + XROWS]),
            'wgt': wgt_b, 'qc': qc128,
        })
    return in_maps


def host_finish(results):
    """Per-core [16, 128, 2*WO] int8 -> [1, 510, 510, 64] NHWC.
    out[hg, 64a+co, pk*WO+w] = pixel (h = 4*hg + 2*pk + a, w, co)."""
    full = np.zeros((8 * RC, WO, COUT), dtype=np.int8)
    for k, r in enumerate(results):
        o = r['out'].reshape(NHG, 2, COUT, 2, WO)           # [hg, a, co, pk, w]
        o = np.transpose(o, (0, 3, 1, 4, 2))                # [hg, pk, a, w, co]
        full[k * RC:(k + 1) * RC] = o.reshape(RC, WO, COUT)
    return np.ascontiguousarray(full[:WO])[None]


def run(inputs, n_cores=8, **kw):
    nc = build_nc(n_cores)
    in_maps = host_prepare(**inputs)[:n_cores]
    res = run_bass_kernel_spmd(nc, in_maps, core_ids=list(range(n_cores)), **kw)
    return host_finish(res.results), res


_CACHED_NC = None


def kernel(x, filt, bias, q_mantissa, exponent, output_zero_point):
    global _CACHED_NC
    if _CACHED_NC is None:
        _CACHED_NC = build_nc(8)
    in_maps = host_prepare(x, filt, bias, q_mantissa, exponent, output_zero_point)
    res = run_bass_kernel_spmd(_CACHED_NC, in_maps, core_ids=list(range(8)))
    return host_finish(res.results)


# revision 37
# speedup vs baseline: 1.0255x; 1.0255x over previous
"""Trainium2 Bass kernel: int8 3x3 VALID conv (1,512,512,32)->(1,510,510,64)
with TFLite fixed-point requantization, SPMD over 8 NeuronCores (output rows).

v7 design:
- Minimal host->device I/O: x ships once per core as int8 [66,32,512];
  the A/B row-phase packings are produced by two strided gather DMAs per
  block and converted int8->bf16 on chip (converts spread over DVE/ACT/
  Pool at [128,512] granularity so the first matmul waits on one group).
- Per half-group: two 1-bank psum tiles (A and B row-pair); 6 matmuls
  ordered j-outer with B first, so consecutive matmuls share stationary
  weights and B's bank closes early.
- Requant (per-channel out = sat(round(acc*sc + zb))): B halves on DVE
  tensor_scalar, A halves on ACT activation -- two independent pipelines
  (separate psum + output tiles, so they never serialize); the final
  half-group stores its two halves via separate DMAs on SP/Pool to
  shorten the tail.
"""
import numpy as np
import ml_dtypes

import concourse.mybir as mybir
import concourse.tile as tile_mod
import concourse.bacc as bacc
from concourse.bass_utils import run_bass_kernel_spmd
from concourse.tile import TileContext
from concourse.ap import AP
from concourse.vector_clock import ScopedClock


def _patched_drain_and_barrier(self, tick_clock, wait_clock):
    # workaround: split the Tile kernel-tail drain into single-wait drains
    # (1 sync-wait per CTRL inst), distributed round-robin across engine
    # queues so they wait in parallel instead of serializing on SP.
    drain_inst = self.nc.sync.drain()
    wait_clock.add_sem_waits(
        drain_inst.ins, ScopedClock({None: tick_clock.global_clock})
    )
    si = drain_inst.ins.sync_info
    if si is not None and si.on_wait and len(si.on_wait) > 1:
        # early-firing clocks first so each engine's drain ladder retires
        # during the final out-DMA wait; DMA-queue clocks (which fire at
        # the last store) land at the ladder ends
        waits = sorted(si.on_wait,
                       key=lambda w: 'DMA' in (getattr(w, 'ant_name', '') or ''))
        drain_inst.ins.sync_info = mybir.SyncInfo(
            on_wait=[waits[0]], on_update=si.on_update
        )
        engines = [self.nc.sync, self.nc.gpsimd, self.nc.vector,
                   self.nc.scalar, self.nc.tensor]
        for i, w in enumerate(waits[1:]):
            d2 = engines[i % len(engines)].drain()
            d2.ins.sync_info = mybir.SyncInfo(on_wait=[w], on_update=[])

    self.nc.all_engine_barrier()
    assert self.sems is not None
    popped = self.nc._tile_sem_poison_stack.pop()
    assert popped is self._sem_poison
    self.nc.clear_and_free_semaphores(list(self.sems.allocated().values()))


tile_mod.TileContext._drain_and_barrier = _patched_drain_and_barrier

dt = mybir.dt
AF = mybir.ActivationFunctionType
OP = mybir.AluOpType

MANT_MAX = 2147418112
H, W, CIN, COUT = 512, 512, 32, 64
WO = 510                     # output width
RC = 64                      # out rows per core
XROWS = 66                   # x rows per core (64 + 2 halo)
NBLK = 4                     # row blocks per core (16 out rows each)
NHG = 16                     # half-groups per core (4 out rows each)


def build_nc(n_cores: int):
    nc = bacc.Bacc('TRN2', target_bir_lowering=False, debug=False,
                   num_devices=n_cores)
    xT = nc.dram_tensor('xT', [XROWS, CIN, W], dt.int8, kind='ExternalInput')
    wgt = nc.dram_tensor('wgt', [128, 3 * 128], dt.bfloat16, kind='ExternalInput')
    qc = nc.dram_tensor('qc', [128, 2], dt.float32, kind='ExternalInput')
    out = nc.dram_tensor('out', [NHG, 128, 2 * WO], dt.int8, kind='ExternalOutput')

    with TileContext(nc) as tc:
        with (
            tc.tile_pool(name='const', bufs=1) as cpool,
            tc.tile_pool(name='xs', bufs=2) as xspool,
            tc.tile_pool(name='ot', bufs=4) as opool,
            tc.tile_pool(name='psa', bufs=4, space='PSUM') as papool,
            tc.tile_pool(name='psb', bufs=3, space='PSUM') as pbpool,
            tc.tile_pool(name='wps', bufs=1, space='PSUM') as wpool,
        ):
            wsb = cpool.tile([128, 3 * 128], dt.bfloat16)
            qsb = cpool.tile([128, 2], dt.float32)
            scr = cpool.tile([128, 512], dt.bfloat16)
            q_sc, q_zb = qsb[:, 0:1], qsb[:, 1:2]

            # head warm-ups, all during the initial DMA wait:
            # - dummy activation pulls the ACT table load (~1.3us) early
            # - dummy matmuls on zeroed data burn the PE clock-ramp
            #   penalty (HAM gate) so real matmuls start at full rate
            #   (dedicated psum pool so no tile dep chains them to DMAs)
            scr2 = cpool.tile([128, 1], dt.float32)
            nc.vector.memset(scr[:], 0.0)
            nc.gpsimd.memset(scr2[:], 0.0)
            nc.scalar.activation(scr2[:], scr2[:], AF.Identity)
            warm = wpool.tile([128, 512], dt.float32)
            for _ in range(4):
                nc.tensor.matmul(warm[:], scr[:, 0:128], scr[:],
                                 start=True, stop=True)

            def xsrc(roff, g0, ng):
                # (p=(q,c), col=(g,w)) <- xT[roff+4(g0+g)+q, c, w]
                return AP(xT, (roff + 4 * g0) * CIN * W,
                          [[CIN * W, 4], [W, 32], [4 * CIN * W, ng], [1, W]])

            for b in range(NBLK):
                sa = xspool.tile([128, 4 * W], dt.int8, tag='sa')
                sb = xspool.tile([128, 4 * W], dt.int8, tag='sb')
                ta = xspool.tile([128, 4 * W], dt.bfloat16, tag='ta')
                tb = xspool.tile([128, 4 * W], dt.bfloat16, tag='tb')
                if b == 0:
                    # critical path first: wgt (ACT queue SEQ overlaps the
                    # table load), then g0 slices of B and A, then the rest
                    nc.scalar.dma_start(wsb[:], wgt[:])
                    nc.gpsimd.dma_start(sb[:, 0:W], xsrc(2, 0, 1))
                    nc.sync.dma_start(sa[:, 0:W], xsrc(0, 0, 1))
                    nc.sync.dma_start(sb[:, W:4 * W], xsrc(2, 1, 3))
                    nc.gpsimd.dma_start(sa[:, W:4 * W], xsrc(0, 1, 3))
                    nc.scalar.dma_start(qsb[:], qc[:])
                else:
                    nc.sync.dma_start(sa[:], xsrc(16 * b, 0, 4))
                    nc.gpsimd.dma_start(sb[:], xsrc(16 * b + 2, 0, 4))
                # int8 -> bf16 at group granularity: tb (feeds the first
                # matmul of each half-group) on DVE; ta on Pool
                for g in range(4):
                    cs = slice(g * W, (g + 1) * W)
                    nc.vector.tensor_scalar(tb[:, cs], sb[:, cs], 0.0, None,
                                            op0=OP.add)
                    nc.gpsimd.tensor_scalar(ta[:, cs], sa[:, cs], 0.0,
                                            None, op0=OP.add)

                for hg in range(4):
                    hgi = 4 * b + hg
                    # independent ACT (A half) and DVE (B half) pipelines:
                    # per-half 1-bank psum tiles; B first so DVE starts a
                    # matmul earlier, and the (B, A) pair per tap j shares
                    # the same stationary weights.
                    ps_b = pbpool.tile([128, 512], dt.float32)
                    ps_a = papool.tile([128, 512], dt.float32)
                    for j in range(3):
                        for t, ps in ((tb, ps_b), (ta, ps_a)):
                            nc.tensor.matmul(
                                ps[:, 0:WO],
                                wsb[:, j * 128:(j + 1) * 128],
                                t[:, hg * W + j: hg * W + j + WO],
                                start=(j == 0), stop=(j == 2))
                    if hgi == NHG - 1:
                        # separate tiles + separate stores so the two
                        # engines' tail chains run fully in parallel
                        ot_a = opool.tile([128, WO], dt.int8, tag='ota')
                        ot_b = opool.tile([128, WO], dt.int8, tag='otb')
                        nc.vector.tensor_scalar(ot_b[:], ps_b[:, 0:WO],
                                                q_sc, q_zb,
                                                op0=OP.mult, op1=OP.add)
                        nc.gpsimd.dma_start(out[hgi, :, WO:2 * WO], ot_b[:])
                        nc.scalar.activation(ot_a[:], ps_a[:, 0:WO],
                                             AF.Identity,
                                             bias=q_zb, scale=q_sc)
                        nc.sync.dma_start(out[hgi, :, 0:WO], ot_a[:])
                    else:
                        # shared ot tile: DVE writes B then ACT writes A;
                        # cross-tile pipelining keeps both engines busy
                        ot = opool.tile([128, 2 * WO], dt.int8, tag='ot')
                        nc.vector.tensor_scalar(ot[:, WO:2 * WO],
                                                ps_b[:, 0:WO],
                                                q_sc, q_zb,
                                                op0=OP.mult, op1=OP.add)
                        nc.scalar.activation(ot[:, 0:WO], ps_a[:, 0:WO],
                                             AF.Identity,
                                             bias=q_zb, scale=q_sc)
                        nc.sync.dma_start(out[hgi], ot[:])
    nc.finalize()
    return nc


def host_prepare(x, filt, bias, q_mantissa, exponent, output_zero_point):
    """Full inputs -> list of per-core in_maps."""
    bf16 = ml_dtypes.bfloat16
    x = np.asarray(x)
    filt = np.asarray(filt)
    bias64 = np.asarray(bias).astype(np.int64)
    qm64 = np.asarray(q_mantissa).astype(np.int64)
    ex64 = np.asarray(exponent).astype(np.int64)
    zp = int(np.asarray(output_zero_point))

    # xT: [H, C, W] int8, padded to 8*64+2 rows
    xpad = np.zeros((8 * RC + 2, CIN, W), dtype=np.int8)
    xpad[:H] = np.ascontiguousarray(x[0].transpose(0, 2, 1))

    # weights: wgt[32q+ci, j, 64a+co] = filt[co, q-a, j, ci] (0 <= q-a <= 2)
    wgtf = np.zeros((128, 3, 128), dtype=np.float32)
    for q in range(4):
        for a in range(2):
            fh = q - a
            if 0 <= fh <= 2:
                wgtf[32 * q:32 * q + 32, :, 64 * a:64 * a + 64] = \
                    filt[:, fh, :, :].transpose(2, 1, 0).astype(np.float32)
    wgt_b = np.ascontiguousarray(wgtf.reshape(128, 384)).astype(bf16)

    # per-channel requant constants (f64 -> f32)
    m = np.where(qm64 < MANT_MAX, (qm64 + (1 << 15)) >> 16, 32767).astype(np.float64)
    s = (15 - ex64).astype(np.float64)
    sc = m * (2.0 ** -s)
    zb = zp + bias64 * sc
    qcv = np.zeros((64, 2), dtype=np.float32)
    qcv[:, 0] = sc
    qcv[:, 1] = zb
    qc128 = np.tile(qcv, (2, 1))

    in_maps = []
    for k in range(8):
        in_maps.append({
            'xT': np.ascontiguousarray(xpad[k * RC: k * RC + XROWS]),
            'wgt': wgt_b, 'qc': qc128,
        })
    return in_maps


def host_finish(results):
    """Per-core [16, 128, 2*WO] int8 -> [1, 510, 510, 64] NHWC.
    out[hg, 64a+co, pk*WO+w] = pixel (h = 4*hg + 2*pk + a, w, co)."""
    full = np.zeros((8 * RC, WO, COUT), dtype=np.int8)
    for k, r in enumerate(results):
        o = r['out'].reshape(NHG, 2, COUT, 2, WO)           # [hg, a, co, pk, w]
        o = np.transpose(o, (0, 3, 1, 4, 2))                # [hg, pk, a, w, co]
        full[k * RC:(k + 1) * RC] = o.reshape(RC, WO, COUT)
    return np.ascontiguousarray(full[:WO])[None]


def run(inputs, n_cores=8, **kw):
    nc = build_nc(n_cores)
    in_maps = host_prepare(**inputs)[:n_cores]
    res = run_bass_kernel_spmd(nc, in_maps, core_ids=list(range(n_cores)), **kw)
    return host_finish(res.results), res


_CACHED_NC = None


def kernel(x, filt, bias, q_mantissa, exponent, output_zero_point):
    global _CACHED_NC
    if _CACHED_NC is None:
        _CACHED_NC = build_nc(8)
    in_maps = host_prepare(x, filt, bias, q_mantissa, exponent, output_zero_point)
    res = run_bass_kernel_spmd(_CACHED_NC, in_maps, core_ids=list(range(8)))
    return host_finish(res.results)
